# revision 27
# baseline (speedup 1.0000x reference)
"""GQA attention with KV cache, tensor-parallel over 8 TRN2 NeuronCores.

Problem shapes (hardcoded): H=32 q-heads, KVH=8 kv-heads, D=128 head_dim,
DIM=4096, T=256 new tokens, MAX_SEQ=8192, pos=4096 (runtime input).

Sharding: head-parallel. Core c owns q-heads 4c..4c+3 and kv-head c:
  wq rows  [c*512:(c+1)*512], wk/wv rows [c*128:(c+1)*128],
  wo cols  [c*512:(c+1)*512], k/v_cache head c.
Each core computes a full (T, DIM) partial of the output projection;
the host sums the 8 partials (the TP all-reduce) and reshapes.

All matmul operands are fp16 (f32 PSUM accumulation): halves HBM traffic
vs f32 with 4x less quantization error than bf16 (all data ranges fit
fp16 comfortably; rel err ~2e-3 vs the 2e-2 budget).

Host-side prep is layout + dtype cast only, and pre-arranges every bulk
tensor PARTITION-MAJOR so each dma_start lands one fat contiguous
descriptor per partition (the v1 kernel was DMA-descriptor-rate-bound:
~15k small descriptors; this cuts it to ~5k large ones).

Per-core dataflow:
  phase A (projections): lhsT = xT k-block (stationary), moving operand
    is the joined weight [wq|wk|wv] (4096, 768): one N=512 matmul (4 q
    heads) + one N=256 matmul (k|v) per (t-tile, k-tile), PSUM-accumulated
    over 32 k-tiles. RoPE applied on the natural [t, d] PSUM with
    free-dim swapped-half views + sign-folded sin tables; roped q/k are
    PE-transposed into qrT/kT ([d, t]); v is evicted into natural blocks.
  phase B (attention, 4 heads batched along the free dim):
    scoresT[s-block] = kT[:, s-block].T @ qrT (2 matmuls N=512),
    probsT = exp(scoresT - 4): one 1024-wide ACT op straight out of PSUM
    (the -4 bias cancels in normalization and keeps the fp16 denominator
    accumulator well inside range),
    pv += v[s-block].T @ probsT (2 matmuls N=512, PSUM accumulation),
    acc_sum += probsT on DVE (fp16 accumulator, 2x DVE mode).
    Denominators, half-pipelined: ones-matmul partition reduce of
    acc_sum, single-pass approx reciprocal (~18 bits), gpsimd
    partition_broadcast; attnT = pv * rinv_bc.
  phase C: out_partial[t-tile, n-chunk] = sum_h attnT_h.T @ woT_h
    (4-matmul PSUM accumulation per chunk, N=512); fp16 partial out.

DMA shaping: xT and wj chunks issue k-interleaved on the sync ring so
arrivals match consumption order; the k/v cache loads are held (via tiny
WAW dummy writes) until the wj stream has landed, and the wo prefetch is
held until qrT is ready, so both stream through phase B's DMA-idle
window instead of starving the phase-A weight stream. Output writes
alternate between the two HWDGE rings. Dummy ident matmuls warm the PE
HAM clock gate at kernel start and across the softmax-denominator tail
(PE-transpose does not count as PE-busy for HAM).
"""

import numpy as np

import concourse.mybir as mybir
import concourse.tile as tile
from concourse import bacc
from concourse.bass_utils import run_bass_kernel_spmd
from concourse.masks import make_identity

H, KVH, D = 32, 8, 128
DIM, T, MAX_SEQ = 4096, 256, 8192
NC_ = 8                      # cores
HL = H // NC_                # local q heads = 4
SCALE = 1.0 / float(np.sqrt(D))
EXP_BIAS = -4.0              # cancels in softmax normalization

F32 = mybir.dt.float32
FP16 = mybir.dt.float16
NP_FP16 = np.float16

_BUILD_CACHE: dict = {}



def _build(pos: int):
    """Trace + compile the per-core program. Same program runs on all 8
    cores (SPMD); only the DRAM input contents differ."""
    S_OLD = pos              # cached tokens
    S = pos + T              # total keys
    NB_OLD = S_OLD // 128    # cached s-blocks (32)
    NB = S // 128            # total s-blocks (34)
    NKT = DIM // 128         # contraction k-tiles (32)
    NT = T // 128            # t-tiles (2)
    WJ = HL * D + 2 * D      # joined weight cols (768)

    nc = bacc.Bacc("TRN2", target_bir_lowering=False, debug=False)

    # All bulk tensors are host-pre-arranged partition-major:
    # row p holds everything partition p will receive, contiguously.
    d_xT = nc.dram_tensor("xTp", (128, NKT * T), FP16, kind="ExternalInput")
    d_wj = nc.dram_tensor("wjp", (128, NKT * WJ), FP16, kind="ExternalInput")
    d_wo = nc.dram_tensor("wop", (128, 8 * HL * 512), FP16,
                          kind="ExternalInput")
    d_kcT = nc.dram_tensor("kcT", (D, S_OLD), FP16, kind="ExternalInput")
    d_vc = nc.dram_tensor("vcp", (128, NB_OLD * D), FP16,
                          kind="ExternalInput")
    d_cq = nc.dram_tensor("cosq4", (T, HL * D), F32, kind="ExternalInput")
    d_sq = nc.dram_tensor("sinq4", (T, HL * D), F32, kind="ExternalInput")
    d_ck = nc.dram_tensor("cosk", (T, D), F32, kind="ExternalInput")
    d_sk = nc.dram_tensor("sink", (T, D), F32, kind="ExternalInput")
    d_out = nc.dram_tensor("out", (T, DIM), FP16, kind="ExternalOutput")

    with tile.TileContext(nc) as tc:
        with (
            tc.tile_pool(name="persist", bufs=1) as pp,
            tc.tile_pool(name="wstream", bufs=1) as wp,
            tc.tile_pool(name="small", bufs=3) as sp,
            tc.tile_pool(name="probs", bufs=4) as prp,
            tc.tile_pool(name="wotile", bufs=8) as wop,
        ):
            # ---- constants ----
            ident = pp.tile([128, 128], FP16, tag="ident")
            scr_i = sp.tile([128, 128], F32, tag="cscr", name="scr_ident")
            make_identity(nc, scr_i[:])
            nc.vector.tensor_copy(ident[:], scr_i[:])
            ones_col = pp.tile([128, 1], FP16, tag="ones")
            scr_o = sp.tile([128, 1], F32, tag="cscr1", name="scr_ones")
            nc.gpsimd.memset(scr_o[:], 1.0)
            nc.vector.tensor_copy(ones_col[:], scr_o[:])
            ebias = pp.tile([128, 1], F32, tag="ebias")
            nc.gpsimd.memset(ebias[:], EXP_BIAS)

            # persistent activations (xT DMAs issue interleaved with wj
            # below so arrivals match k-order consumption)
            xT = pp.tile([128, NKT, T], FP16, tag="xT")
            CH = [1, 1, 2, 4, 4, 4, 4, 4, 4, 4]   # k-tiles per chunk
            kT_all = pp.tile([128, S], FP16, tag="kT")           # [d, s]
            v_all = pp.tile([128, NB * D], FP16, tag="vall")
            # rotary tables (natural [t, d] layout, host-tiled/-scaled)
            cs = {}
            for nm, dt_, w in (("cosq4", d_cq, HL * D), ("sinq4", d_sq, HL * D),
                               ("cosk", d_ck, D), ("sink", d_sk, D)):
                tl = [pp.tile([128, w], F32, tag=f"{nm}{i}",
                              name=f"cs_{nm}{i}") for i in range(NT)]
                for i in range(NT):
                    nc.scalar.dma_start(tl[i][:],
                                        dt_.ap()[i * 128:(i + 1) * 128, :])
                cs[nm] = tl
            qrT = pp.tile([128, HL * T], FP16, tag="qrT")        # [d, h*T+t]
            acc_sum = pp.tile([128, HL * T], FP16, tag="accsum")
            attnT = pp.tile([128, HL * T], FP16, tag="attnT")
            rinv = pp.tile([1, HL * T], F32, tag="rinv")
            rinv_bc = pp.tile([128, HL * T], F32, tag="rinvbc")

            # ================= phase A: QKV projections + RoPE ============
            with (
                tc.tile_pool(name="ps_proj", bufs=1, space="PSUM") as ps_pj,
                tc.tile_pool(name="ps_tr", bufs=2, space="PSUM") as ps_tr,
            ):
                # PE warmup while the first wj/xT chunks are in flight:
                # real matmuls (transpose-mode doesn't count as PE-busy for
                # the HAM clock gate), ~3.4us to reach the 8/8 clock
                for w_ in range(22):
                    pw = ps_tr.tile([128, 128], F32, tag="wa",
                                    name=f"warm{w_}")
                    nc.tensor.matmul(pw[:], ident[:], ident[:],
                                     start=True, stop=True)
                ps_q = [ps_pj.tile([128, HL * D], F32, tag=f"psq{i}",
                                   name=f"ps_q{i}") for i in range(NT)]
                ps_kv = [ps_pj.tile([128, 2 * D], F32, tag=f"pskv{i}",
                                    name=f"ps_kv{i}") for i in range(NT)]
                wj_sb = wp.tile([128, NKT, WJ], FP16, tag="wj")
                koff = 0
                for ci, sz in enumerate(CH):
                    ring = nc.sync
                    ring.dma_start(
                        xT[:, koff:koff + sz, :],
                        d_xT.ap()[:, koff * T:(koff + sz) * T]
                        .rearrange("p (k t) -> p k t", t=T))
                    ring.dma_start(
                        wj_sb[:, koff:koff + sz, :],
                        d_wj.ap()[:, koff * WJ:(koff + sz) * WJ]
                        .rearrange("p (c w) -> p c w", w=WJ))
                    koff += sz
                # cache loads held until the wj stream has fully landed
                # (tiny WAW dummy writes reading the last wj chunk), so the
                # phase-A weight stream gets the full DMA bandwidth
                nc.vector.tensor_copy(kT_all[0:1, 0:8], wj_sb[0:1, NKT - 1, 0:8])
                nc.vector.tensor_copy(v_all[0:1, 0:8], wj_sb[0:1, NKT - 1, 0:8])
                nc.sync.dma_start(kT_all[:, 0:S_OLD], d_kcT.ap()[:, :])
                nc.scalar.dma_start(v_all[:, 0:NB_OLD * D], d_vc.ap()[:, :])
                for k in range(NKT):
                    for i in range(NT):
                        nc.tensor.matmul(
                            ps_q[i][:], xT[:, k, i * 128:(i + 1) * 128],
                            wj_sb[:, k, 0:HL * D],
                            start=(k == 0), stop=(k == NKT - 1))
                        nc.tensor.matmul(
                            ps_kv[i][:], xT[:, k, i * 128:(i + 1) * 128],
                            wj_sb[:, k, HL * D:WJ],
                            start=(k == 0), stop=(k == NKT - 1))

                def rope_swap_view(ap, nh):
                    """View of ap ([128, nh*128]) with the two half-D lanes
                    swapped inside each head: col h*128 + 64*a + b reads
                    h*128 + 64*(1-a) + b."""
                    v = ap.rearrange("p (h a b) -> p h a b", a=2, b=64)
                    return v[:, :, ::-1, :]

                # q RoPE by head pair (pair 0 = qrT half 0, which the
                # first scores matmuls consume) then k, then v
                for hp in range(2):
                    for i in range(NT):
                        ps = slice(hp * 2 * D, (hp + 1) * 2 * D)
                        m1 = sp.tile([128, 2 * D], FP16, tag="m1")
                        nc.vector.tensor_mul(m1[:], ps_q[i][:, ps],
                                             cs["cosq4"][i][:, ps])
                        m2 = sp.tile([128, 2 * D], FP16, tag="m2")
                        nc.vector.tensor_mul(
                            m2[:], rope_swap_view(ps_q[i][:, ps], 2),
                            cs["sinq4"][i][:, ps])
                        qr_nat = sp.tile([128, 2 * D], FP16, tag="qrnat")
                        nc.vector.tensor_add(qr_nat[:], m1[:], m2[:])
                        for hh in range(2):
                            h = hp * 2 + hh
                            p = ps_tr.tile([128, 128], FP16, tag="tr")
                            nc.tensor.transpose(
                                p[:], qr_nat[:, hh * 128:(hh + 1) * 128],
                                ident[:])
                            nc.vector.tensor_copy(
                                qrT[:, h * T + i * 128: h * T + (i + 1) * 128],
                                p[:])
                # raw k|v eviction only: the k RoPE + transpose and the
                # v block placement are deferred into early phase B (their
                # consumers are s-blocks 32/33 at the END of the s-loop),
                # so the PE queue reaches the first score matmuls sooner
                kv_sb = [None] * NT
                for i in range(NT):
                    kv_sb[i] = pp.tile([128, 2 * D], FP16, tag=f"kvsb{i}",
                                       name=f"kv_sb{i}")
                    nc.vector.tensor_copy(kv_sb[i][:], ps_kv[i][:])

            # wo prefetch, held until qrT is ready so it streams during
            # phase B's DMA-idle window instead of competing with wj
            wo_ch = []
            for n in range(DIM // 512):
                w_sb = wop.tile([128, HL, 512], FP16, tag="wot",
                                name=f"wo_n{n}")
                nc.vector.tensor_copy(w_sb[0:1, 0, 0:8], qrT[0:1, 0:8])
                nc.sync.dma_start(
                    w_sb[:],
                    d_wo.ap()[:, n * HL * 512:(n + 1) * HL * 512]
                    .rearrange("p (h m) -> p h m", m=512))
                wo_ch.append(w_sb)

            # ================= phase B: attention =========================
            def rope_swap_sb(ap):
                v = ap.rearrange("p (a b) -> p a b", a=2, b=64)
                return v[:, ::-1, :]

            with tc.tile_pool(name="ps_pv", bufs=1, space="PSUM") as ps_pv:
                pv = ps_pv.tile([128, HL * T], F32, tag="pv")
                with (
                    tc.tile_pool(name="ps_sc", bufs=2, space="PSUM") as ps_sc,
                    tc.tile_pool(name="ps_ktr", bufs=2, space="PSUM") as ps_kt,
                ):
                    for s in range(NB):
                        if s == 3:
                            # deferred k RoPE + v placement, interleaved
                            # into the s-loop's PE/DVE slack
                            for i in range(NT):
                                km1 = sp.tile([128, D], FP16, tag="km1")
                                nc.vector.tensor_mul(
                                    km1[:], kv_sb[i][:, 0:D], cs["cosk"][i][:])
                                km2 = sp.tile([128, D], FP16, tag="km2")
                                nc.vector.tensor_mul(
                                    km2[:], rope_swap_sb(kv_sb[i][:, 0:D]),
                                    cs["sink"][i][:])
                                kr_nat = sp.tile([128, D], FP16, tag="krnat")
                                nc.vector.tensor_add(kr_nat[:], km1[:], km2[:])
                                p = ps_kt.tile([128, 128], FP16, tag="ktr")
                                nc.tensor.transpose(p[:], kr_nat[:], ident[:])
                                nc.vector.tensor_copy(
                                    kT_all[:, S_OLD + i * 128:
                                           S_OLD + (i + 1) * 128], p[:])
                                nc.vector.tensor_copy(
                                    v_all[:, (NB_OLD + i) * D:
                                          (NB_OLD + i + 1) * D],
                                    kv_sb[i][:, D:2 * D])
                        sc = ps_sc.tile([128, HL * T], F32, tag="sc")
                        for half in range(2):
                            nc.tensor.matmul(
                                sc[:, half * 512:(half + 1) * 512],
                                kT_all[:, s * 128:(s + 1) * 128],
                                qrT[:, half * 512:(half + 1) * 512],
                                start=True, stop=True)
                        pb = prp.tile([128, HL * T], FP16, tag="pb")
                        nc.scalar.activation(
                            pb[:], sc[:], mybir.ActivationFunctionType.Exp,
                            bias=ebias[:])
                        if s == 0:
                            nc.vector.tensor_copy(acc_sum[:], pb[:])
                        else:
                            nc.vector.tensor_add(acc_sum[:], acc_sum[:], pb[:])
                        for half in range(2):
                            hs = slice(half * 512, (half + 1) * 512)
                            nc.tensor.matmul(
                                pv[:, hs],
                                v_all[:, s * D:(s + 1) * D],
                                pb[:, hs],
                                start=(s == 0), stop=(s == NB - 1))

                # softmax denominators + normalization: ones-matmul
                # partition reduce, fast approx reciprocal on the [1, 1024]
                # row, gpsimd broadcast, then scale pv
                with (
                    tc.tile_pool(name="ps_post", bufs=1, space="PSUM") as psp,
                    tc.tile_pool(name="ps_warm", bufs=2, space="PSUM") as pw_p,
                ):
                    # denominators half-pipelined: colsum -> recip -> bcast
                    # -> scale per 512-wide half, engines overlapping
                    sm = psp.tile([1, HL * T], F32, tag="sm")
                    for half in range(2):
                        hs = slice(half * 512, (half + 1) * 512)
                        nc.tensor.matmul(
                            sm[:, hs], ones_col[:], acc_sum[:, hs],
                            start=True, stop=True)
                        nc.vector.reciprocal_approx_fast(
                            rinv[:, hs], sm[:, hs])
                        nc.gpsimd.partition_broadcast(
                            rinv_bc[:, hs], rinv[:, hs])
                    def tview(ap, i):
                        v = ap.rearrange("p (h t) -> p h t", t=T)
                        return v[:, :, i * 128:(i + 1) * 128]
                    for i in range(NT):
                        nc.vector.tensor_mul(
                            tview(attnT[:], i), tview(pv[:], i),
                            tview(rinv_bc[:], i))
                    # keep the PE busy (HAM warm) across the serial tail
                    # with real matmuls (transpose-mode doesn't count)
                    for w_ in range(8):
                        pw = pw_p.tile([128, 512], F32, tag="wk",
                                       name=f"wk{w_}")
                        nc.tensor.matmul(pw[:], ident[:], qrT[:, 0:512],
                                         start=True, stop=True)
                    for w_ in range(3):
                        pw = pw_p.tile([128, 512], F32, tag="wk",
                                       name=f"wkb{w_}")
                        nc.tensor.matmul(pw[:], ident[:], attnT[:, 0:512],
                                         start=True, stop=True)

            # ================= phase C: output projection =================
            # n in halves of 4 so the per-(i,h) stationary attnT slice is
            # loaded once for 4 matmuls (LDW dedupe collapses them); the
            # 4 po accumulators per (half, i) fit PSUM comfortably.
            with (
                tc.tile_pool(name="ps_wo", bufs=8, space="PSUM") as ps_wo,
                tc.tile_pool(name="obp", bufs=6) as obp,
            ):
                for nh in range(2):
                    for i in range(NT):
                        pos_ = [ps_wo.tile([128, 512], F32, tag="po",
                                           name=f"po_{nh}_{i}_{n_}")
                                for n_ in range(4)]
                        for h in range(HL):
                            for n_ in range(4):
                                n = nh * 4 + n_
                                nc.tensor.matmul(
                                    pos_[n_][:],
                                    attnT[:, h * T + i * 128:
                                          h * T + (i + 1) * 128],
                                    wo_ch[n][:, h, :],
                                    start=(h == 0), stop=(h == HL - 1))
                        ob = obp.tile([128, 4 * 512], FP16, tag="ob",
                                      name=f"ob_{nh}_{i}")
                        for n_ in range(4):
                            if n_ % 2 == 0:
                                nc.vector.tensor_copy(
                                    ob[:, n_ * 512:(n_ + 1) * 512],
                                    pos_[n_][:])
                            else:
                                nc.scalar.copy(
                                    ob[:, n_ * 512:(n_ + 1) * 512],
                                    pos_[n_][:])
                        # halves on both HWDGE rings: parallel queue drain
                        nc.sync.dma_start(
                            d_out.ap()[i * 128:(i + 1) * 128,
                                       nh * 2048:nh * 2048 + 1024],
                            ob[:, 0:1024])
                        nc.scalar.dma_start(
                            d_out.ap()[i * 128:(i + 1) * 128,
                                       nh * 2048 + 1024:(nh + 1) * 2048],
                            ob[:, 1024:2048])

    nc.compile()
    return nc


def _pmajor(a, np_, inner):
    """(np_*128, inner) row-blocked array -> (128, np_*inner) partition-
    major: out[p, j*inner:(j+1)*inner] = a[j*128 + p, :]."""
    return np.ascontiguousarray(
        a.reshape(np_, 128, inner).transpose(1, 0, 2).reshape(
            128, np_ * inner))


def _prep_inputs(x, cos, sin, wq, wk, wv, wo, k_cache, v_cache, pos):
    """Host-side shard + layout/dtype prep (no arithmetic beyond scaling
    the rotary tables). Returns in_maps for the 8 cores."""
    f = np.float32
    pos = int(pos)
    x2d = np.asarray(x, dtype=f).reshape(T, DIM)
    xTp = _pmajor(np.ascontiguousarray(x2d.T).astype(NP_FP16), DIM // 128, T)
    cos = np.asarray(cos, dtype=np.float64)
    sin = np.asarray(sin, dtype=np.float64)
    sgn = np.concatenate([-np.ones(D // 2), np.ones(D // 2)])
    cosq4 = np.ascontiguousarray(np.tile(cos * SCALE, (1, HL)), dtype=f)
    sinq4 = np.ascontiguousarray(np.tile(sin * sgn * SCALE, (1, HL)), dtype=f)
    cosk = np.ascontiguousarray(cos, dtype=f)
    sink = np.ascontiguousarray(sin * sgn, dtype=f)
    wq = np.asarray(wq, dtype=f)
    wk = np.asarray(wk, dtype=f)
    wv = np.asarray(wv, dtype=f)
    wo = np.asarray(wo, dtype=f)
    k_cache = np.asarray(k_cache, dtype=f)
    v_cache = np.asarray(v_cache, dtype=f)
    in_maps = []
    for c in range(NC_):
        wj = np.concatenate([
            wq[c * HL * D:(c + 1) * HL * D, :].T,
            wk[c * D:(c + 1) * D, :].T,
            wv[c * D:(c + 1) * D, :].T], axis=1)          # (DIM, WJ)
        woT = wo[:, c * HL * D:(c + 1) * HL * D].T        # (HL*D, DIM)
        # (128, 8*HL*512): partition d-within-head, then [n, h, m]
        wop_ = woT.astype(NP_FP16).reshape(HL, 128, 8, 512).transpose(
            1, 2, 0, 3).reshape(128, 8 * HL * 512)
        in_maps.append({
            "xTp": xTp,
            "wjp": _pmajor(wj.astype(NP_FP16), DIM // 128, HL * D + 2 * D),
            "wop": np.ascontiguousarray(wop_),
            "kcT": np.ascontiguousarray(
                k_cache[c, :pos, :].T.astype(NP_FP16)),
            "vcp": _pmajor(v_cache[c, :pos, :].astype(NP_FP16),
                           pos // 128, D),
            "cosq4": cosq4, "sinq4": sinq4, "cosk": cosk, "sink": sink,
        })
    return in_maps


def run(trace=False, **inputs):
    """Build (cached), run on 8 cores, reduce. Returns (out, results)."""
    pos = int(inputs["pos"])
    if pos not in _BUILD_CACHE:
        _BUILD_CACHE[pos] = _build(pos)
    nc = _BUILD_CACHE[pos]
    in_maps = _prep_inputs(**inputs)
    res = run_bass_kernel_spmd(
        nc, in_maps, core_ids=list(range(NC_)), trace=trace)
    part = np.stack([np.asarray(r["out"], np.float32)
                     for r in res.results])               # (8, T, DIM)
    out = part.sum(axis=0, dtype=np.float32).reshape(1, T, DIM)
    return out, res


def kernel(**inputs):
    out, _ = run(trace=False, **inputs)
    return out


# revision 28
# speedup vs baseline: 1.1675x; 1.1675x over previous
"""GQA attention with KV cache, tensor-parallel over 8 TRN2 NeuronCores.

Problem shapes (hardcoded): H=32 q-heads, KVH=8 kv-heads, D=128 head_dim,
DIM=4096, T=256 new tokens, MAX_SEQ=8192, pos=4096 (runtime input).

Sharding: head-parallel. Core c owns q-heads 4c..4c+3 and kv-head c:
  wq rows  [c*512:(c+1)*512], wk/wv rows [c*128:(c+1)*128],
  wo cols  [c*512:(c+1)*512], k/v_cache head c.
Each core computes a full (T, DIM) partial of the output projection;
the host sums the 8 partials (the TP all-reduce) and reshapes.

All matmul operands are fp16 (f32 PSUM accumulation): halves HBM traffic
vs f32 with 4x less quantization error than bf16 (all data ranges fit
fp16 comfortably; rel err ~2e-3 vs the 2e-2 budget).

Host-side prep is layout + dtype cast only, and pre-arranges every bulk
tensor PARTITION-MAJOR so each dma_start lands one fat contiguous
descriptor per partition (the v1 kernel was DMA-descriptor-rate-bound:
~15k small descriptors; this cuts it to ~5k large ones).

Per-core dataflow:
  phase A (projections): lhsT = xT k-block (stationary), moving operand
    is the joined weight [wq|wk|wv] (4096, 768): one N=512 matmul (4 q
    heads) + one N=256 matmul (k|v) per (t-tile, k-tile), PSUM-accumulated
    over 32 k-tiles. RoPE applied on the natural [t, d] PSUM with
    free-dim swapped-half views + sign-folded sin tables; roped q/k are
    PE-transposed into qrT/kT ([d, t]); v is evicted into natural blocks.
  phase B (attention, 4 heads batched along the free dim):
    scoresT[s-block] = kT[:, s-block].T @ qrT (2 matmuls N=512),
    probsT = exp(scoresT - 4): one 1024-wide ACT op straight out of PSUM
    (the -4 bias cancels in normalization and keeps the fp16 denominator
    accumulator well inside range),
    pv += v[s-block].T @ probsT (2 matmuls N=512, PSUM accumulation),
    acc_sum += probsT on DVE (fp16 accumulator, 2x DVE mode).
    Denominators, half-pipelined: ones-matmul partition reduce of
    acc_sum, single-pass approx reciprocal (~18 bits), gpsimd
    partition_broadcast; attnT = pv * rinv_bc.
  phase C: out_partial[t-tile, n-chunk] = sum_h attnT_h.T @ woT_h
    (4-matmul PSUM accumulation per chunk, N=512); fp16 partial out.

DMA shaping: xT and wj chunks issue k-interleaved on the sync ring so
arrivals match consumption order; the k/v cache loads are held (via tiny
WAW dummy writes) until the wj stream has landed, and the wo prefetch is
held until qrT is ready, so both stream through phase B's DMA-idle
window instead of starving the phase-A weight stream. Output writes
alternate between the two HWDGE rings. Dummy ident matmuls warm the PE
HAM clock gate at kernel start and across the softmax-denominator tail
(PE-transpose does not count as PE-busy for HAM).
"""

import numpy as np

import concourse.mybir as mybir
import concourse.tile as tile
from concourse import bacc
from concourse.bass_utils import run_bass_kernel_spmd
from concourse.masks import make_identity

H, KVH, D = 32, 8, 128
DIM, T, MAX_SEQ = 4096, 256, 8192
NC_ = 8                      # cores
HL = H // NC_                # local q heads = 4
SCALE = 1.0 / float(np.sqrt(D))
EXP_BIAS = -4.0              # cancels in softmax normalization

F32 = mybir.dt.float32
FP16 = mybir.dt.float16
NP_FP16 = np.float16

_BUILD_CACHE: dict = {}



def _build(pos: int):
    """Trace + compile the per-core program. Same program runs on all 8
    cores (SPMD); only the DRAM input contents differ."""
    S_OLD = pos              # cached tokens
    S = pos + T              # total keys
    NB_OLD = S_OLD // 128    # cached s-blocks (32)
    NB = S // 128            # total s-blocks (34)
    NKT = DIM // 128         # contraction k-tiles (32)
    NT = T // 128            # t-tiles (2)
    WJ = HL * D + 2 * D      # joined weight cols (768)

    nc = bacc.Bacc("TRN2", target_bir_lowering=False, debug=False)

    # All bulk tensors are host-pre-arranged partition-major:
    # row p holds everything partition p will receive, contiguously.
    d_xT = nc.dram_tensor("xTp", (128, NKT * T), FP16, kind="ExternalInput")
    d_wj = nc.dram_tensor("wjp", (128, NKT * WJ), FP16, kind="ExternalInput")
    d_wo = nc.dram_tensor("wop", (128, 8 * HL * 512), FP16,
                          kind="ExternalInput")
    d_kcT = nc.dram_tensor("kcT", (D, S_OLD), FP16, kind="ExternalInput")
    d_vc = nc.dram_tensor("vcp", (128, NB_OLD * D), FP16,
                          kind="ExternalInput")
    d_cq = nc.dram_tensor("cosq4", (T, HL * D), F32, kind="ExternalInput")
    d_sq = nc.dram_tensor("sinq4", (T, HL * D), F32, kind="ExternalInput")
    d_ck = nc.dram_tensor("cosk", (T, D), F32, kind="ExternalInput")
    d_sk = nc.dram_tensor("sink", (T, D), F32, kind="ExternalInput")
    d_out = nc.dram_tensor("out", (T, DIM), FP16, kind="ExternalOutput")

    with tile.TileContext(nc) as tc:
        with (
            tc.tile_pool(name="persist", bufs=1) as pp,
            tc.tile_pool(name="wstream", bufs=1) as wp,
            tc.tile_pool(name="small", bufs=3) as sp,
            tc.tile_pool(name="probs", bufs=4) as prp,
            tc.tile_pool(name="wotile", bufs=8) as wop,
        ):
            # ---- constants ----
            ident = pp.tile([128, 128], FP16, tag="ident")
            scr_i = sp.tile([128, 128], F32, tag="cscr", name="scr_ident")
            make_identity(nc, scr_i[:])
            nc.vector.tensor_copy(ident[:], scr_i[:])
            ones_col = pp.tile([128, 1], FP16, tag="ones")
            scr_o = sp.tile([128, 1], F32, tag="cscr1", name="scr_ones")
            nc.gpsimd.memset(scr_o[:], 1.0)
            nc.vector.tensor_copy(ones_col[:], scr_o[:])
            ebias = pp.tile([128, 1], F32, tag="ebias")
            nc.gpsimd.memset(ebias[:], EXP_BIAS)

            # persistent activations (xT DMAs issue interleaved with wj
            # below so arrivals match k-order consumption)
            xT = pp.tile([128, NKT, T], FP16, tag="xT")
            CH = [1, 1, 2, 4, 4, 4, 4, 4, 4, 4]   # k-tiles per chunk
            kT_all = pp.tile([128, S], FP16, tag="kT")           # [d, s]
            v_all = pp.tile([128, NB * D], FP16, tag="vall")
            # rotary tables (natural [t, d] layout, host-tiled/-scaled)
            cs = {}
            for nm, dt_, w in (("cosq4", d_cq, HL * D), ("sinq4", d_sq, HL * D),
                               ("cosk", d_ck, D), ("sink", d_sk, D)):
                tl = [pp.tile([128, w], F32, tag=f"{nm}{i}",
                              name=f"cs_{nm}{i}") for i in range(NT)]
                for i in range(NT):
                    nc.scalar.dma_start(tl[i][:],
                                        dt_.ap()[i * 128:(i + 1) * 128, :])
                cs[nm] = tl
            qrT = pp.tile([128, HL * T], FP16, tag="qrT")        # [d, h*T+t]
            acc_sum = pp.tile([128, HL * T], FP16, tag="accsum")
            attnT = pp.tile([128, HL * T], FP16, tag="attnT")
            rinv = pp.tile([1, HL * T], F32, tag="rinv")
            rinv_bc = pp.tile([128, HL * T], F32, tag="rinvbc")

            # ================= phase A: QKV projections + RoPE ============
            with (
                tc.tile_pool(name="ps_proj", bufs=1, space="PSUM") as ps_pj,
                tc.tile_pool(name="ps_tr", bufs=2, space="PSUM") as ps_tr,
            ):
                # PE warmup while the first wj/xT chunks are in flight:
                # real matmuls (transpose-mode doesn't count as PE-busy for
                # the HAM clock gate), ~3.4us to reach the 8/8 clock
                for w_ in range(22):
                    pw = ps_tr.tile([128, 128], F32, tag="wa",
                                    name=f"warm{w_}")
                    nc.tensor.matmul(pw[:], ident[:], ident[:],
                                     start=True, stop=True)
                ps_q = [ps_pj.tile([128, HL * D], F32, tag=f"psq{i}",
                                   name=f"ps_q{i}") for i in range(NT)]
                ps_kv = [ps_pj.tile([128, 2 * D], F32, tag=f"pskv{i}",
                                    name=f"ps_kv{i}") for i in range(NT)]
                wj_sb = wp.tile([128, NKT, WJ], FP16, tag="wj")
                koff = 0
                for ci, sz in enumerate(CH):
                    ring = nc.sync
                    ring.dma_start(
                        xT[:, koff:koff + sz, :],
                        d_xT.ap()[:, koff * T:(koff + sz) * T]
                        .rearrange("p (k t) -> p k t", t=T))
                    ring.dma_start(
                        wj_sb[:, koff:koff + sz, :],
                        d_wj.ap()[:, koff * WJ:(koff + sz) * WJ]
                        .rearrange("p (c w) -> p c w", w=WJ))
                    koff += sz
                # cache loads held until the wj stream has fully landed
                # (tiny WAW dummy writes reading the last wj chunk), so the
                # phase-A weight stream gets the full DMA bandwidth
                nc.vector.tensor_copy(kT_all[0:1, 0:8], wj_sb[0:1, NKT - 1, 0:8])
                nc.vector.tensor_copy(v_all[0:1, 0:8], wj_sb[0:1, NKT - 1, 0:8])
                nc.sync.dma_start(kT_all[:, 0:S_OLD], d_kcT.ap()[:, :])
                nc.scalar.dma_start(v_all[:, 0:NB_OLD * D], d_vc.ap()[:, :])
                for k in range(NKT):
                    for i in range(NT):
                        nc.tensor.matmul(
                            ps_q[i][:], xT[:, k, i * 128:(i + 1) * 128],
                            wj_sb[:, k, 0:HL * D],
                            start=(k == 0), stop=(k == NKT - 1))
                        nc.tensor.matmul(
                            ps_kv[i][:], xT[:, k, i * 128:(i + 1) * 128],
                            wj_sb[:, k, HL * D:WJ],
                            start=(k == 0), stop=(k == NKT - 1))

                def rope_swap_view(ap, nh):
                    """View of ap ([128, nh*128]) with the two half-D lanes
                    swapped inside each head: col h*128 + 64*a + b reads
                    h*128 + 64*(1-a) + b."""
                    v = ap.rearrange("p (h a b) -> p h a b", a=2, b=64)
                    return v[:, :, ::-1, :]

                # q RoPE by head pair (pair 0 = qrT half 0, which the
                # first scores matmuls consume) then k, then v
                for hp in range(2):
                    for i in range(NT):
                        ps = slice(hp * 2 * D, (hp + 1) * 2 * D)
                        m1 = sp.tile([128, 2 * D], FP16, tag="m1")
                        nc.vector.tensor_mul(m1[:], ps_q[i][:, ps],
                                             cs["cosq4"][i][:, ps])
                        m2 = sp.tile([128, 2 * D], FP16, tag="m2")
                        nc.vector.tensor_mul(
                            m2[:], rope_swap_view(ps_q[i][:, ps], 2),
                            cs["sinq4"][i][:, ps])
                        qr_nat = sp.tile([128, 2 * D], FP16, tag="qrnat")
                        nc.vector.tensor_add(qr_nat[:], m1[:], m2[:])
                        for hh in range(2):
                            h = hp * 2 + hh
                            p = ps_tr.tile([128, 128], FP16, tag="tr")
                            nc.tensor.transpose(
                                p[:], qr_nat[:, hh * 128:(hh + 1) * 128],
                                ident[:])
                            nc.vector.tensor_copy(
                                qrT[:, h * T + i * 128: h * T + (i + 1) * 128],
                                p[:])
                # raw k|v eviction only: the k RoPE + transpose and the
                # v block placement are deferred into early phase B (their
                # consumers are s-blocks 32/33 at the END of the s-loop),
                # so the PE queue reaches the first score matmuls sooner
                kv_sb = [None] * NT
                for i in range(NT):
                    kv_sb[i] = pp.tile([128, 2 * D], FP16, tag=f"kvsb{i}",
                                       name=f"kv_sb{i}")
                    nc.vector.tensor_copy(kv_sb[i][:], ps_kv[i][:])

            # wo prefetch, held until qrT is ready so it streams during
            # phase B's DMA-idle window instead of competing with wj
            wo_ch = []
            for n in range(DIM // 512):
                w_sb = wop.tile([128, HL, 512], FP16, tag="wot",
                                name=f"wo_n{n}")
                nc.vector.tensor_copy(w_sb[0:1, 0, 0:8], qrT[0:1, 0:8])
                nc.sync.dma_start(
                    w_sb[:],
                    d_wo.ap()[:, n * HL * 512:(n + 1) * HL * 512]
                    .rearrange("p (h m) -> p h m", m=512))
                wo_ch.append(w_sb)

            # ================= phase B: attention =========================
            def rope_swap_sb(ap):
                v = ap.rearrange("p (a b) -> p a b", a=2, b=64)
                return v[:, ::-1, :]

            with tc.tile_pool(name="ps_pv", bufs=1, space="PSUM") as ps_pv:
                pv = ps_pv.tile([128, HL * T], F32, tag="pv")
                with (
                    tc.tile_pool(name="ps_sc", bufs=2, space="PSUM") as ps_sc,
                    tc.tile_pool(name="ps_ktr", bufs=2, space="PSUM") as ps_kt,
                ):
                    for s in range(NB):
                        if s == 3:
                            # deferred k RoPE + v placement, interleaved
                            # into the s-loop's PE/DVE slack
                            for i in range(NT):
                                km1 = sp.tile([128, D], FP16, tag="km1")
                                nc.vector.tensor_mul(
                                    km1[:], kv_sb[i][:, 0:D], cs["cosk"][i][:])
                                km2 = sp.tile([128, D], FP16, tag="km2")
                                nc.vector.tensor_mul(
                                    km2[:], rope_swap_sb(kv_sb[i][:, 0:D]),
                                    cs["sink"][i][:])
                                kr_nat = sp.tile([128, D], FP16, tag="krnat")
                                nc.vector.tensor_add(kr_nat[:], km1[:], km2[:])
                                p = ps_kt.tile([128, 128], FP16, tag="ktr")
                                nc.tensor.transpose(p[:], kr_nat[:], ident[:])
                                nc.vector.tensor_copy(
                                    kT_all[:, S_OLD + i * 128:
                                           S_OLD + (i + 1) * 128], p[:])
                                nc.vector.tensor_copy(
                                    v_all[:, (NB_OLD + i) * D:
                                          (NB_OLD + i + 1) * D],
                                    kv_sb[i][:, D:2 * D])
                        sc = ps_sc.tile([128, HL * T], F32, tag="sc")
                        for half in range(2):
                            nc.tensor.matmul(
                                sc[:, half * 512:(half + 1) * 512],
                                kT_all[:, s * 128:(s + 1) * 128],
                                qrT[:, half * 512:(half + 1) * 512],
                                start=True, stop=True)
                        pb = prp.tile([128, HL * T], FP16, tag="pb")
                        nc.scalar.activation(
                            pb[:], sc[:], mybir.ActivationFunctionType.Exp,
                            bias=ebias[:])
                        if s == 0:
                            nc.vector.tensor_copy(acc_sum[:], pb[:])
                        else:
                            nc.vector.tensor_add(acc_sum[:], acc_sum[:], pb[:])
                        for half in range(2):
                            hs = slice(half * 512, (half + 1) * 512)
                            nc.tensor.matmul(
                                pv[:, hs],
                                v_all[:, s * D:(s + 1) * D],
                                pb[:, hs],
                                start=(s == 0), stop=(s == NB - 1))

                # softmax denominators + normalization: ones-matmul
                # partition reduce, fast approx reciprocal on the [1, 1024]
                # row, gpsimd broadcast, then scale pv
                with (
                    tc.tile_pool(name="ps_post", bufs=1, space="PSUM") as psp,
                    tc.tile_pool(name="ps_warm", bufs=2, space="PSUM") as pw_p,
                ):
                    # denominators half-pipelined: colsum -> recip -> bcast
                    # -> scale per 512-wide half, engines overlapping
                    sm = psp.tile([1, HL * T], F32, tag="sm")
                    for half in range(2):
                        hs = slice(half * 512, (half + 1) * 512)
                        nc.tensor.matmul(
                            sm[:, hs], ones_col[:], acc_sum[:, hs],
                            start=True, stop=True)
                        nc.vector.reciprocal_approx_fast(
                            rinv[:, hs], sm[:, hs])
                        nc.gpsimd.partition_broadcast(
                            rinv_bc[:, hs], rinv[:, hs])
                    for half in range(2):
                        hs = slice(half * 512, (half + 1) * 512)
                        nc.vector.tensor_mul(
                            attnT[:, hs], pv[:, hs], rinv_bc[:, hs])
                    # keep the PE busy (HAM warm) across the serial tail
                    # with real matmuls (transpose-mode doesn't count)
                    for w_ in range(8):
                        pw = pw_p.tile([128, 512], F32, tag="wk",
                                       name=f"wk{w_}")
                        nc.tensor.matmul(pw[:], ident[:], qrT[:, 0:512],
                                         start=True, stop=True)
                    for w_ in range(3):
                        pw = pw_p.tile([128, 512], F32, tag="wk",
                                       name=f"wkb{w_}")
                        nc.tensor.matmul(pw[:], ident[:], attnT[:, 0:512],
                                         start=True, stop=True)

            # ================= phase C: output projection =================
            # n in halves of 4 so the per-(i,h) stationary attnT slice is
            # loaded once for 4 matmuls (LDW dedupe collapses them); the
            # 4 po accumulators per (half, i) fit PSUM comfortably.
            with (
                tc.tile_pool(name="ps_wo", bufs=8, space="PSUM") as ps_wo,
                tc.tile_pool(name="obp", bufs=6) as obp,
            ):
                for nh in range(2):
                    for i in range(NT):
                        pos_ = [ps_wo.tile([128, 512], F32, tag="po",
                                           name=f"po_{nh}_{i}_{n_}")
                                for n_ in range(4)]
                        for h in range(HL):
                            for n_ in range(4):
                                n = nh * 4 + n_
                                nc.tensor.matmul(
                                    pos_[n_][:],
                                    attnT[:, h * T + i * 128:
                                          h * T + (i + 1) * 128],
                                    wo_ch[n][:, h, :],
                                    start=(h == 0), stop=(h == HL - 1))
                        ob = obp.tile([128, 4 * 512], FP16, tag="ob",
                                      name=f"ob_{nh}_{i}")
                        for n_ in range(4):
                            if n_ % 2 == 0:
                                nc.vector.tensor_copy(
                                    ob[:, n_ * 512:(n_ + 1) * 512],
                                    pos_[n_][:])
                            else:
                                nc.scalar.copy(
                                    ob[:, n_ * 512:(n_ + 1) * 512],
                                    pos_[n_][:])
                        # halves on both HWDGE rings: parallel queue drain
                        nc.sync.dma_start(
                            d_out.ap()[i * 128:(i + 1) * 128,
                                       nh * 2048:nh * 2048 + 1024],
                            ob[:, 0:1024])
                        nc.scalar.dma_start(
                            d_out.ap()[i * 128:(i + 1) * 128,
                                       nh * 2048 + 1024:(nh + 1) * 2048],
                            ob[:, 1024:2048])

    nc.compile()
    return nc


def _pmajor(a, np_, inner):
    """(np_*128, inner) row-blocked array -> (128, np_*inner) partition-
    major: out[p, j*inner:(j+1)*inner] = a[j*128 + p, :]."""
    return np.ascontiguousarray(
        a.reshape(np_, 128, inner).transpose(1, 0, 2).reshape(
            128, np_ * inner))


def _prep_inputs(x, cos, sin, wq, wk, wv, wo, k_cache, v_cache, pos):
    """Host-side shard + layout/dtype prep (no arithmetic beyond scaling
    the rotary tables). Returns in_maps for the 8 cores."""
    f = np.float32
    pos = int(pos)
    x2d = np.asarray(x, dtype=f).reshape(T, DIM)
    xTp = _pmajor(np.ascontiguousarray(x2d.T).astype(NP_FP16), DIM // 128, T)
    cos = np.asarray(cos, dtype=np.float64)
    sin = np.asarray(sin, dtype=np.float64)
    sgn = np.concatenate([-np.ones(D // 2), np.ones(D // 2)])
    cosq4 = np.ascontiguousarray(np.tile(cos * SCALE, (1, HL)), dtype=f)
    sinq4 = np.ascontiguousarray(np.tile(sin * sgn * SCALE, (1, HL)), dtype=f)
    cosk = np.ascontiguousarray(cos, dtype=f)
    sink = np.ascontiguousarray(sin * sgn, dtype=f)
    wq = np.asarray(wq, dtype=f)
    wk = np.asarray(wk, dtype=f)
    wv = np.asarray(wv, dtype=f)
    wo = np.asarray(wo, dtype=f)
    k_cache = np.asarray(k_cache, dtype=f)
    v_cache = np.asarray(v_cache, dtype=f)
    in_maps = []
    for c in range(NC_):
        wj = np.concatenate([
            wq[c * HL * D:(c + 1) * HL * D, :].T,
            wk[c * D:(c + 1) * D, :].T,
            wv[c * D:(c + 1) * D, :].T], axis=1)          # (DIM, WJ)
        woT = wo[:, c * HL * D:(c + 1) * HL * D].T        # (HL*D, DIM)
        # (128, 8*HL*512): partition d-within-head, then [n, h, m]
        wop_ = woT.astype(NP_FP16).reshape(HL, 128, 8, 512).transpose(
            1, 2, 0, 3).reshape(128, 8 * HL * 512)
        in_maps.append({
            "xTp": xTp,
            "wjp": _pmajor(wj.astype(NP_FP16), DIM // 128, HL * D + 2 * D),
            "wop": np.ascontiguousarray(wop_),
            "kcT": np.ascontiguousarray(
                k_cache[c, :pos, :].T.astype(NP_FP16)),
            "vcp": _pmajor(v_cache[c, :pos, :].astype(NP_FP16),
                           pos // 128, D),
            "cosq4": cosq4, "sinq4": sinq4, "cosk": cosk, "sink": sink,
        })
    return in_maps


def run(trace=False, **inputs):
    """Build (cached), run on 8 cores, reduce. Returns (out, results)."""
    pos = int(inputs["pos"])
    if pos not in _BUILD_CACHE:
        _BUILD_CACHE[pos] = _build(pos)
    nc = _BUILD_CACHE[pos]
    in_maps = _prep_inputs(**inputs)
    res = run_bass_kernel_spmd(
        nc, in_maps, core_ids=list(range(NC_)), trace=trace)
    part = np.stack([np.asarray(r["out"], np.float32)
                     for r in res.results])               # (8, T, DIM)
    out = part.sum(axis=0, dtype=np.float32).reshape(1, T, DIM)
    return out, res


def kernel(**inputs):
    out, _ = run(trace=False, **inputs)
    return out
